# revision 3
# baseline (speedup 1.0000x reference)
"""Trainium2 Bass kernel for nn_LocalWalk (local correlation -> exp -> dense scatter).

Math (reference): att[b,q,(p1,p2)] = <query[b,:,q], keys[b,:,q+off]> / 0.1 for a
25x25 window; out-of-image window entries (zero-padded corr == 0) map to
exp(-10) scattered into column 0; everything else is exp(att) scattered into a
dense (B, HW, HW) map, returned as (B, HW, H, W).

Key observation: out[b, k, h, w] = exp(10 * <q[b,:,h,w], k[b,:,kh,kw]>) iff
|kh-h|<=12 and |kw-w|<=12, else 0 -- i.e. each output channel k is a <=25x25
patch around pixel k, plus a constant border map on channel 0.

Sharding: 8 cores = (batch b in [0,4)) x (kh half in {0,1}). Each core computes
its 2048 output channels with TensorE matmuls:
  per pair of key rows (M = 2*64 = 128 lhsT columns), stream 26 zero-padded
  query rows (88 cols each) as rhs -> PSUM (128, 26*88) in 6 chunks of <=440.
  A second constant matmul accumulates -1e37 outside each channel's horizontal
  band / invalid vertical rows, so ScalarE exp(10*x) from PSUM directly
  produces masked patch values (0 outside the band). Per (pair, t) row the
  destination row r = 2*j + t is core-offset independent, so one SPMD program
  serves all cores; image-boundary clipping lands in a 12-row junk margin that
  the host slices off during unsharding.
"""

import numpy as np

B, C, H, W = 4, 128, 64, 64
P = 25
R = 12  # P//2
NEG = np.float32(-1.0e37)
N_CORES = 8
NPAIR = 16          # 32 key rows per core, 2 per matmul group
NT = 26             # query rows streamed per pair
NR = 56             # output rows per channel in core layout (2*15 + 26)
QW = 88             # padded query width
CHUNK = 5           # t-rows per matmul (N = 5*88 = 440 <= 512 fp32 PSUM bank)

_CACHED = {}


def _host_constants():
    m = np.arange(128)
    kw = m % 64
    w_mask = np.zeros((90, 128), dtype=np.float32)
    for c in range(QW):
        w_mask[c, :] = np.where((c >= kw) & (c <= kw + 24), 0.0, NEG)
    w_mask[88, :] = np.where(m < 64, NEG, 0.0)    # kappa=0: t=25 invalid
    w_mask[89, :] = np.where(m >= 64, NEG, 0.0)   # kappa=1: t=0 invalid
    r_mask = np.zeros((90, NT * QW), dtype=np.float32)
    for t in range(NT):
        r_mask[np.arange(QW), t * QW + np.arange(QW)] = 1.0
    r_mask[88, 25 * QW:] = 1.0
    r_mask[89, :QW] = 1.0

    e10 = np.float32(np.exp(np.float32(-10.0)))
    hv = np.minimum(np.arange(64) + R, 63) - np.maximum(np.arange(64) - R, 0) + 1
    n_valid = hv[:, None] * hv[None, :]
    corr_const = ((625 - n_valid) * np.float64(e10)).astype(np.float32)
    return w_mask, r_mask, corr_const


def _build_nc():
    from contextlib import ExitStack

    import concourse.mybir as mybir
    import concourse.tile as tile
    from concourse import bacc

    f32 = mybir.dt.float32
    nc = bacc.Bacc("TRN2", target_bir_lowering=False, debug=False,
                   enable_asserts=False, num_devices=N_CORES)

    keys_in = nc.dram_tensor("keys_in", [128, 32 * 64], f32, kind="ExternalInput").ap()
    q_in = nc.dram_tensor("q_in", [128, NR * QW], f32, kind="ExternalInput").ap()
    wm_in = nc.dram_tensor("wm_in", [90, 128], f32, kind="ExternalInput").ap()
    rm_in = nc.dram_tensor("rm_in", [90, NT * QW], f32, kind="ExternalInput").ap()
    out = nc.dram_tensor("out", [2048, NR, 64], f32, kind="ExternalOutput").ap()

    with tile.TileContext(nc) as tc, ExitStack() as ctx:
        const = ctx.enter_context(tc.tile_pool(name="const", bufs=1))
        psum = ctx.enter_context(tc.tile_pool(name="psum", bufs=8, space="PSUM"))
        tpool = ctx.enter_context(tc.tile_pool(name="tout", bufs=4))

        keys_sb = const.tile([128, 32 * 64], f32)
        nc.sync.dma_start(keys_sb[:, :], keys_in[:, :])
        q_sb = const.tile([128, NR * QW], f32)
        nc.sync.dma_start(q_sb[:, :], q_in[:, :])
        w_sb = const.tile([90, 128], f32)
        nc.sync.dma_start(w_sb[:, :], wm_in[:, :])
        r_sb = const.tile([90, NT * QW], f32)
        nc.sync.dma_start(r_sb[:, :], rm_in[:, :])
        zero_sb = const.tile([128, 30, 64], f32)
        nc.vector.memset(zero_sb[:, :, :], 0.0)

        exp_t = mybir.ActivationFunctionType.Exp

        for j in range(NPAIR):
            pm = []
            for m in range(6):
                nt = CHUNK if m < 5 else 1
                p_t = psum.tile([128, CHUNK, QW], f32, tag="pm", name=f"pm{j}_{m}")
                nc.tensor.matmul(
                    p_t[:, :nt, :],
                    lhsT=keys_sb[:, 128 * j: 128 * (j + 1)],
                    rhs=q_sb[:, (2 * j + CHUNK * m) * QW: (2 * j + CHUNK * m + nt) * QW],
                    start=True, stop=False,
                )
                pm.append(p_t)
            for m in range(6):
                nt = CHUNK if m < 5 else 1
                nc.tensor.matmul(
                    pm[m][:, :nt, :],
                    lhsT=w_sb[:, :],
                    rhs=r_sb[:, CHUNK * m * QW: (CHUNK * m + nt) * QW],
                    start=False, stop=True,
                )
            t_sb = tpool.tile([128, NT, 64], f32, tag="T", name=f"t{j}")
            for m in range(6):
                nt = CHUNK if m < 5 else 1
                nc.scalar.activation(
                    t_sb[:, CHUNK * m: CHUNK * m + nt, :],
                    pm[m][:, :nt, R: R + 64],
                    exp_t, scale=10.0,
                )
            nc.sync.dma_start(out[128 * j: 128 * (j + 1), 2 * j: 2 * j + NT, :],
                              t_sb[:, :, :])
            if j > 0:
                nc.sync.dma_start(out[128 * j: 128 * (j + 1), 0: 2 * j, :],
                                  zero_sb[:, : 2 * j, :])
            if j < NPAIR - 1:
                nc.sync.dma_start(out[128 * j: 128 * (j + 1), 2 * j + NT: NR, :],
                                  zero_sb[:, : 30 - 2 * j, :])
    nc.compile()
    return nc


def kernel(query: np.ndarray, keys: np.ndarray) -> np.ndarray:
    from concourse.bass_utils import run_bass_kernel_spmd

    query = np.ascontiguousarray(np.asarray(query, dtype=np.float32))
    keys = np.ascontiguousarray(np.asarray(keys, dtype=np.float32))
    w_mask, r_mask, corr_const = _host_constants()

    if "nc" not in _CACHED:
        _CACHED["nc"] = _build_nc()
    nc = _CACHED["nc"]

    in_maps = []
    for core in range(N_CORES):
        b, half = core >> 1, core & 1
        kh0 = 32 * half
        k_blk = np.ascontiguousarray(
            keys[b][:, kh0: kh0 + 32, :].reshape(128, 32 * 64))
        qp = np.zeros((128, NR, QW), dtype=np.float32)
        g_lo, g_hi = max(0, kh0 - R), min(H, kh0 + 44)
        qp[:, g_lo - (kh0 - R): g_hi - (kh0 - R), R: R + W] = query[b][:, g_lo: g_hi, :]
        in_maps.append({
            "keys_in": k_blk,
            "q_in": qp.reshape(128, NR * QW),
            "wm_in": w_mask,
            "rm_in": r_mask,
        })

    _CACHED["in_maps"] = in_maps
    res = run_bass_kernel_spmd(nc, in_maps, core_ids=list(range(N_CORES)))

    full = np.zeros((B, H * W, H, W), dtype=np.float32)
    for core in range(N_CORES):
        b, half = core >> 1, core & 1
        kh0 = 32 * half
        oc = res.results[core]["out"].reshape(2048, NR, 64)
        h_lo, h_hi = max(0, kh0 - R), min(H, kh0 + 44)
        full[b, 2048 * half: 2048 * (half + 1), h_lo:h_hi, :] = \
            oc[:, h_lo - kh0 + R: h_hi - kh0 + R, :]
    for b in range(B):
        full[b, 0] += corr_const
    return full


# revision 6
# speedup vs baseline: 40.7211x; 40.7211x over previous
"""Trainium2 Bass kernel for nn_LocalWalk (local correlation -> exp -> dense scatter).

Math (reference): att[b,q,(p1,p2)] = <query[b,:,q], keys[b,:,q+off]> / 0.1 for a
25x25 window; out-of-image window entries (zero-padded corr == 0) map to
exp(-10) scattered into column 0; everything else is exp(att) scattered into a
dense (B, HW, HW) map, returned as (B, HW, H, W).

Key observation: out[b, k, h, w] = exp(10 * <q[b,:,h,w], k[b,:,kh,kw]>) iff
|kh-h|<=12 and |kw-w|<=12, else 0 -- i.e. each output channel k is a <=25x25
patch around pixel k, plus a constant border map on channel 0.

Sharding: 8 cores = (batch b in [0,4)) x (kh half in {0,1}). Each core computes
its 2048 output channels with TensorE matmuls:
  per pair of key rows (M = 2*64 = 128 lhsT columns), stream 26 zero-padded
  query rows (88 cols each) as rhs -> PSUM (128, 26*88) in 6 chunks of <=440.
  A second constant matmul accumulates -1e37 outside each channel's horizontal
  band / invalid vertical rows, so ScalarE exp(10*x) from PSUM directly
  produces masked patch values (0 outside the band). Per (pair, t) row the
  destination row r = 2*j + t is core-offset independent, so one SPMD program
  serves all cores; image-boundary clipping lands in a 12-row junk margin that
  the host slices off during unsharding.
"""

import numpy as np

B, C, H, W = 4, 128, 64, 64
P = 25
R = 12  # P//2
NEG = np.float32(-1.0e37)
N_CORES = 8
NPAIR = 16          # 32 key rows per core, 2 per matmul group
NT = 26             # query rows streamed per pair
NR = 56             # output rows per channel in core layout (2*15 + 26)
QW = 88             # padded query width
CHUNK = 5           # t-rows per matmul (N = 5*88 = 440 <= 512 fp32 PSUM bank)

_CACHED = {}


def _host_constants():
    m = np.arange(128)
    kw = m % 64
    w_mask = np.zeros((90, 128), dtype=np.float32)
    for c in range(QW):
        w_mask[c, :] = np.where((c >= kw) & (c <= kw + 24), 0.0, NEG)
    w_mask[88, :] = np.where(m < 64, NEG, 0.0)    # kappa=0: t=25 invalid
    w_mask[89, :] = np.where(m >= 64, NEG, 0.0)   # kappa=1: t=0 invalid
    r_mask = np.zeros((90, NT * QW), dtype=np.float32)
    for t in range(NT):
        r_mask[np.arange(QW), t * QW + np.arange(QW)] = 1.0
    r_mask[88, 25 * QW:] = 1.0
    r_mask[89, :QW] = 1.0

    e10 = np.float32(np.exp(np.float32(-10.0)))
    hv = np.minimum(np.arange(64) + R, 63) - np.maximum(np.arange(64) - R, 0) + 1
    n_valid = hv[:, None] * hv[None, :]
    corr_const = ((625 - n_valid) * np.float64(e10)).astype(np.float32)
    return w_mask, r_mask, corr_const


def _build_nc(t_bufs=4, split_tdma=False, interleave=False):
    from contextlib import ExitStack

    import concourse.mybir as mybir
    import concourse.tile as tile
    from concourse import bacc

    f32 = mybir.dt.float32
    nc = bacc.Bacc("TRN2", target_bir_lowering=False, debug=False,
                   enable_asserts=False, num_devices=N_CORES)

    keys_in = nc.dram_tensor("keys_in", [128, 32 * 64], f32, kind="ExternalInput").ap()
    q_in = nc.dram_tensor("q_in", [128, NR * QW], f32, kind="ExternalInput").ap()
    wm_in = nc.dram_tensor("wm_in", [90, 128], f32, kind="ExternalInput").ap()
    rm_in = nc.dram_tensor("rm_in", [90, NT * QW], f32, kind="ExternalInput").ap()
    out = nc.dram_tensor("out", [2048, NR, 64], f32, kind="ExternalOutput").ap()

    with tile.TileContext(nc) as tc, ExitStack() as ctx:
        const = ctx.enter_context(tc.tile_pool(name="const", bufs=1))
        psum = ctx.enter_context(tc.tile_pool(name="psum", bufs=8, space="PSUM"))
        tpool = ctx.enter_context(tc.tile_pool(name="tout", bufs=t_bufs))

        keys_sb = const.tile([128, 32 * 64], f32)
        nc.sync.dma_start(keys_sb[:, :], keys_in[:, :])
        q_sb = const.tile([128, NR * QW], f32)
        nc.sync.dma_start(q_sb[:, :], q_in[:, :])
        w_sb = const.tile([90, 128], f32)
        nc.sync.dma_start(w_sb[:, :], wm_in[:, :])
        r_sb = const.tile([90, NT * QW], f32)
        nc.sync.dma_start(r_sb[:, :], rm_in[:, :])
        zero_sb = const.tile([128, 30, 64], f32)
        nc.vector.memset(zero_sb[:, :, :], 0.0)

        exp_t = mybir.ActivationFunctionType.Exp

        for j in range(NPAIR):
            pm = []

            def data_mm(m):
                nt = CHUNK if m < 5 else 1
                p_t = psum.tile([128, CHUNK, QW], f32, tag="pm", name=f"pm{j}_{m}")
                nc.tensor.matmul(
                    p_t[:, :nt, :],
                    lhsT=keys_sb[:, 128 * j: 128 * (j + 1)],
                    rhs=q_sb[:, (2 * j + CHUNK * m) * QW: (2 * j + CHUNK * m + nt) * QW],
                    start=True, stop=False,
                )
                pm.append(p_t)

            def mask_mm(m):
                nt = CHUNK if m < 5 else 1
                nc.tensor.matmul(
                    pm[m][:, :nt, :],
                    lhsT=w_sb[:, :],
                    rhs=r_sb[:, CHUNK * m * QW: (CHUNK * m + nt) * QW],
                    start=False, stop=True,
                )

            if interleave:
                for m in range(6):
                    data_mm(m)
                    mask_mm(m)
            else:
                for m in range(6):
                    data_mm(m)
                for m in range(6):
                    mask_mm(m)
            t_sb = tpool.tile([128, NT, 64], f32, tag="T", name=f"t{j}")
            for m in range(6):
                nt = CHUNK if m < 5 else 1
                nc.scalar.activation(
                    t_sb[:, CHUNK * m: CHUNK * m + nt, :],
                    pm[m][:, :nt, R: R + 64],
                    exp_t, scale=10.0,
                )
            if split_tdma:
                nc.sync.dma_start(out[128 * j: 128 * (j + 1), 2 * j: 2 * j + 13, :],
                                  t_sb[:, :13, :])
                nc.sync.dma_start(out[128 * j: 128 * (j + 1), 2 * j + 13: 2 * j + NT, :],
                                  t_sb[:, 13:, :])
            else:
                nc.sync.dma_start(out[128 * j: 128 * (j + 1), 2 * j: 2 * j + NT, :],
                                  t_sb[:, :, :])
            if j > 0:
                nc.sync.dma_start(out[128 * j: 128 * (j + 1), 0: 2 * j, :],
                                  zero_sb[:, : 2 * j, :])
            if j < NPAIR - 1:
                nc.sync.dma_start(out[128 * j: 128 * (j + 1), 2 * j + NT: NR, :],
                                  zero_sb[:, : 30 - 2 * j, :])
    nc.compile()
    return nc


def kernel(query: np.ndarray, keys: np.ndarray) -> np.ndarray:
    from concourse.bass_utils import run_bass_kernel_spmd

    query = np.ascontiguousarray(np.asarray(query, dtype=np.float32))
    keys = np.ascontiguousarray(np.asarray(keys, dtype=np.float32))
    w_mask, r_mask, corr_const = _host_constants()

    if "nc" not in _CACHED:
        _CACHED["nc"] = _build_nc()
    nc = _CACHED["nc"]

    in_maps = []
    for core in range(N_CORES):
        b, half = core >> 1, core & 1
        kh0 = 32 * half
        k_blk = np.ascontiguousarray(
            keys[b][:, kh0: kh0 + 32, :].reshape(128, 32 * 64))
        qp = np.zeros((128, NR, QW), dtype=np.float32)
        g_lo, g_hi = max(0, kh0 - R), min(H, kh0 + 44)
        qp[:, g_lo - (kh0 - R): g_hi - (kh0 - R), R: R + W] = query[b][:, g_lo: g_hi, :]
        in_maps.append({
            "keys_in": k_blk,
            "q_in": qp.reshape(128, NR * QW),
            "wm_in": w_mask,
            "rm_in": r_mask,
        })

    _CACHED["in_maps"] = in_maps
    res = run_bass_kernel_spmd(nc, in_maps, core_ids=list(range(N_CORES)))

    full = np.zeros((B, H * W, H, W), dtype=np.float32)
    for core in range(N_CORES):
        b, half = core >> 1, core & 1
        kh0 = 32 * half
        oc = res.results[core]["out"].reshape(2048, NR, 64)
        h_lo, h_hi = max(0, kh0 - R), min(H, kh0 + 44)
        full[b, 2048 * half: 2048 * (half + 1), h_lo:h_hi, :] = \
            oc[:, h_lo - kh0 + R: h_hi - kh0 + R, :]
    for b in range(B):
        full[b, 0] += corr_const
    return full
